# revision 22
# baseline (speedup 1.0000x reference)
# PointNet++ feature-propagation (three_nn + three_interpolate + shared MLP)
# Trainium2 Bass/Tile kernel, 8 NeuronCores, data-parallel over batch.
#
# Per batch (n=4096 unknown, m=1024 known, C2=512, C1=256):
#  1) ONE bf16 matmul (K=24 rows of triple-bf16 splits) computes
#     D' = 2u.k - |k|^2 - |u|^2 = -d2 directly in PSUM (error ~2e-6 abs);
#     Max/MaxIndex scan PSUM directly -> top-3 neighbors + their -d2.
#  2) weights from -d2 (clamped), no refine pass needed.
#  3) G = W0a^T @ known_feats is precomputed per batch ([256, m]); its
#     transpose rows (bf16, 512B) are staged to HBM and gathered per point
#     (3 neighbors in one DGE gather), halving gather bytes vs raw feats.
#  4) interp contribution = sum_k w_k * g_k is injected into the MLP1 PSUM
#     via identity matmuls; MLP1 = relu(W0b^T uf + inject), MLP2 as usual.
#
# The two batches are software-pipelined and the post-scan work is cut into
# quarter-batch (1024-point) groups so the DVE (which owns the unavoidable
# Max/MaxIndex scans, ~150us) never stalls and the post-scan tail is short.
import numpy as np
from contextlib import ExitStack

import concourse.bass as bass
import concourse.bacc as bacc
import concourse.tile as tile
import concourse.mybir as mybir
from concourse.masks import make_identity

AP = bass.AP
dt = mybir.dt
Alu = mybir.AluOpType
ACTF = mybir.ActivationFunctionType

B_FULL = 16
N_CORES = 8
NB = 2            # batches per core
N = 4096
M = 1024
C1 = 256
C2 = 512
D0 = 256
D1 = 256

NCH = N // 128    # 32 i-chunks
MCH = M // 128    # 8 j-chunks
QCH = 8           # i-chunks per group (quarter batch)
QN = QCH * 128    # 1024 points per group
NG = NCH // QCH   # 4 groups per batch
KROWS = 24
D2_FLOOR = 1e-7   # clamp for d2 (reference adds 1e-8; matmul err ~2e-6)


def _bf16_split3(ve, pool, x_ap, shape, tagp):
    """bf16 (hi, lo, mid) with hi+lo+mid ~= x."""
    xh = pool.tile(list(shape), dt.bfloat16, tag=tagp + "_h")
    xl = pool.tile(list(shape), dt.bfloat16, tag=tagp + "_l")
    xm = pool.tile(list(shape), dt.bfloat16, tag=tagp + "_m")
    r1 = pool.tile(list(shape), dt.float32, tag=tagp + "_r1")
    r2 = pool.tile(list(shape), dt.float32, tag=tagp + "_r2")
    ve.tensor_copy(xh[:], x_ap)
    ve.tensor_sub(r1[:], x_ap, xh[:])
    ve.tensor_copy(xl[:], r1[:])
    ve.tensor_sub(r2[:], r1[:], xl[:])
    ve.tensor_copy(xm[:], r2[:])
    return xh, xl, xm


def _v(t_ap, dims, off=0):
    """AP over t_ap's tensor with explicit [stride, count] dims (dims[0] = partition dim)."""
    return AP(t_ap.tensor, t_ap.offset + off, dims)


def build_nc(nb=NB):
    nc = bacc.Bacc("TRN2", target_bir_lowering=False, debug=False)

    unknown_h = nc.dram_tensor("unknown", [nb, N, 3], dt.float32, kind="ExternalInput")
    known_h = nc.dram_tensor("known", [nb, M, 3], dt.float32, kind="ExternalInput")
    uf_h = nc.dram_tensor("unknow_feats", [nb, C1, N], dt.float32, kind="ExternalInput")
    kf_h = nc.dram_tensor("known_feats", [nb, C2, M], dt.float32, kind="ExternalInput")
    w0_h = nc.dram_tensor("W0", [C1 + C2, D0], dt.float32, kind="ExternalInput")
    w1_h = nc.dram_tensor("W1", [D0, D1], dt.float32, kind="ExternalInput")
    out_h = nc.dram_tensor("out", [nb, D1, N], dt.float32, kind="ExternalOutput")

    GROUPS = [[(0, 8), (8, 8), (16, 8), (24, 8)] for _ in range(nb)]
    GROUPS[nb - 1] = [(0, 8), (8, 8), (16, 8), (24, 4), (28, 4)]

    gt_h = [nc.dram_tensor(f"gt{b}", [M, D0], dt.bfloat16) for b in range(nb)]
    wr_h = [[nc.dram_tensor(f"wr{b}_{gi}", [3 * sz * 128], dt.bfloat16)
             for gi, (c0, sz) in enumerate(GROUPS[b])]
            for b in range(nb)]

    with tile.TileContext(nc) as tc, ExitStack() as ctx:
        const = ctx.enter_context(tc.tile_pool(name="const", bufs=1))
        kfp = ctx.enter_context(tc.tile_pool(name="kfp", bufs=2))
        kf32p = ctx.enter_context(tc.tile_pool(name="kf32p", bufs=1))
        gtp = ctx.enter_context(tc.tile_pool(name="gtp", bufs=1))
        prep = ctx.enter_context(tc.tile_pool(name="prep", bufs=2))
        sp = ctx.enter_context(tc.tile_pool(name="split", bufs=2))
        sel = ctx.enter_context(tc.tile_pool(name="sel", bufs=2))
        wts = ctx.enter_context(tc.tile_pool(name="wts", bufs=6))
        wts4 = ctx.enter_context(tc.tile_pool(name="wts4", bufs=2))
        gat = ctx.enter_context(tc.tile_pool(name="gat", bufs=2))
        gat4 = ctx.enter_context(tc.tile_pool(name="gat4", bufs=2))
        gwp = ctx.enter_context(tc.tile_pool(name="gwp", bufs=2))
        gwp4 = ctx.enter_context(tc.tile_pool(name="gwp4", bufs=2))
        wbp = ctx.enter_context(tc.tile_pool(name="wbp", bufs=2))
        wbp4 = ctx.enter_context(tc.tile_pool(name="wbp4", bufs=2))
        mlpp = ctx.enter_context(tc.tile_pool(name="mlpp", bufs=2))
        mlpp4 = ctx.enter_context(tc.tile_pool(name="mlpp4", bufs=1))
        ps_d = ctx.enter_context(tc.tile_pool(name="ps_d", bufs=2, space="PSUM"))
        ps_mm = ctx.enter_context(tc.tile_pool(name="ps_mm", bufs=2, space="PSUM"))
        ps_trb = ctx.enter_context(tc.tile_pool(name="ps_trb", bufs=1, space="PSUM"))
        ps_tru = ctx.enter_context(tc.tile_pool(name="ps_tru", bufs=1, space="PSUM"))

        # ---------------- constants ----------------
        ident_b = const.tile([128, 128], dt.bfloat16, tag="idb")
        make_identity(nc, ident_b[:])
        ident_u = const.tile([128, 128], dt.float16, tag="idu")
        make_identity(nc, ident_u[:])

        w0_sb = const.tile([128, 6, D0], dt.bfloat16, tag="w0")
        w1_sb = const.tile([128, 2, D1], dt.bfloat16, tag="w1")
        for ci in range(6):
            wtmp = kf32p.tile([128, D0], dt.float32, tag="wstg")
            nc.sync.dma_start(wtmp[:], w0_h.ap()[128 * ci:128 * ci + 128, :])
            nc.scalar.copy(w0_sb[:, ci, :], wtmp[:])
        for ci in range(2):
            wtmp = kf32p.tile([128, D1], dt.float32, tag="wstg")
            nc.sync.dma_start(wtmp[:], w1_h.ap()[128 * ci:128 * ci + 128, :])
            nc.scalar.copy(w1_sb[:, ci, :], wtmp[:])

        lhs_alls, rhs_alls = [], []
        valls, mialls = [], []
        wtiles = {}

        def do_prep(b):
            ve = nc.gpsimd
            # ---- known prep -> rhs_all
            kw = prep.tile([128, MCH, 3], dt.float32, tag="kw")
            nc.sync.dma_start(
                kw[:], AP(known_h, b * M * 3, [[3, 128], [3 * 128, MCH], [1, 3]])
            )
            k2 = prep.tile([128, MCH, 3], dt.float32, tag="k2")
            ve.tensor_scalar_mul(k2[:], kw[:], 2.0)
            k2h, k2l, k2m = _bf16_split3(ve, sp, k2[:], [128, MCH, 3], "k2")
            sq = prep.tile([128, MCH, 3], dt.float32, tag="ksq")
            nc.scalar.square(sq[:], kw[:])
            s_f = prep.tile([128, MCH], dt.float32, tag="ks")
            ve.tensor_add(s_f[:], sq[:, :, 0], sq[:, :, 1])
            ve.tensor_add(s_f[:], s_f[:], sq[:, :, 2])
            ns = prep.tile([128, MCH], dt.float32, tag="kns")
            ve.tensor_scalar_mul(ns[:], s_f[:], -1.0)
            nsh, nsl, nsm = _bf16_split3(ve, sp, ns[:], [128, MCH], "kns")

            # rows: 0-2 uh|2kh, 3-5 uh|2kl, 6-8 ul|2kh, 9-11 ul|2kl,
            #       12-14 uh|2km, 15-17 um|2kh, 18-20 1|-(skh,skl,skm),
            #       21-23 -(suh,sul,sum)|1
            kch = prep.tile([128, MCH, 32], dt.bfloat16, tag="kch")
            for (r0, src) in ((0, k2h), (3, k2l), (6, k2h), (9, k2l), (12, k2m), (15, k2h)):
                nc.scalar.copy(kch[:, :, r0:r0 + 3], src[:])
            nc.scalar.copy(kch[:, :, 18], nsh[:])
            nc.scalar.copy(kch[:, :, 19], nsl[:])
            nc.scalar.copy(kch[:, :, 20], nsm[:])
            ve.memset(kch[:, :, 21:24], 1.0)
            rhs_all = prep.tile([KROWS, M], dt.bfloat16, tag="rhs_all")
            for t in range(MCH):
                pst = ps_trb.tile([48, 128], dt.bfloat16, tag="trb")
                nc.tensor.transpose(pst[:KROWS, :], kch[:, t, :KROWS], ident_b[:])
                nc.scalar.copy(rhs_all[:, 128 * t:128 * t + 128], pst[:KROWS, :])

            # ---- unknown prep -> lhs_all
            uw = prep.tile([128, NCH, 3], dt.float32, tag="uw")
            nc.sync.dma_start(
                uw[:], AP(unknown_h, b * N * 3, [[3, 128], [3 * 128, NCH], [1, 3]])
            )
            uh, ul, um = _bf16_split3(ve, sp, uw[:], [128, NCH, 3], "u")
            usq = prep.tile([128, NCH, 3], dt.float32, tag="usq")
            nc.scalar.square(usq[:], uw[:])
            su = prep.tile([128, NCH], dt.float32, tag="us")
            ve.tensor_add(su[:], usq[:, :, 0], usq[:, :, 1])
            ve.tensor_add(su[:], su[:], usq[:, :, 2])
            nsu = prep.tile([128, NCH], dt.float32, tag="uns")
            ve.tensor_scalar_mul(nsu[:], su[:], -1.0)
            nsuh, nsul, nsum_ = _bf16_split3(ve, sp, nsu[:], [128, NCH], "uns")

            uch = prep.tile([128, NCH, 32], dt.bfloat16, tag="uch")
            for (r0, src) in ((0, uh), (3, uh), (6, ul), (9, ul), (12, uh), (15, um)):
                nc.scalar.copy(uch[:, :, r0:r0 + 3], src[:])
            ve.memset(uch[:, :, 18:21], 1.0)
            nc.scalar.copy(uch[:, :, 21], nsuh[:])
            nc.scalar.copy(uch[:, :, 22], nsul[:])
            nc.scalar.copy(uch[:, :, 23], nsum_[:])
            lhs_all = prep.tile([KROWS, N], dt.bfloat16, tag="lhs_all")
            for t in range(NCH):
                pst = ps_trb.tile([48, 128], dt.bfloat16, tag="trb")
                nc.tensor.transpose(pst[:KROWS, :], uch[:, t, :KROWS], ident_b[:])
                nc.scalar.copy(lhs_all[:, 128 * t:128 * t + 128], pst[:KROWS, :])
            lhs_alls.append(lhs_all)
            rhs_alls.append(rhs_all)

        def do_weights(b, gi, c0, sz):
            # small DVE ops right after this group's scans: weights + idx list
            vall, miall = valls[b], mialls[b]
            wp = wts if sz == QCH else wts4
            tsl = slice(c0, c0 + sz)
            d23 = wp.tile([128, sz, 3], dt.float32, tag="d23")
            nc.vector.tensor_scalar(
                d23[:], vall[:, tsl, 0:3], -1.0, D2_FLOOR, op0=Alu.mult, op1=Alu.max
            )
            r3 = wp.tile([128, sz, 3], dt.float32, tag="r3")
            nc.vector.reciprocal(r3[:], d23[:])
            z = wp.tile([128, sz], dt.float32, tag="z")
            nc.vector.tensor_reduce(z[:], r3[:], axis=mybir.AxisListType.X, op=Alu.add)
            iz = wp.tile([128, sz], dt.float32, tag="iz")
            nc.vector.reciprocal(iz[:], z[:])
            w3f = wp.tile([128, sz, 3], dt.float32, tag="w3f")
            nc.vector.tensor_mul(w3f[:], r3[:], iz[:].to_broadcast([128, sz, 3]))
            # k-major bf16: w3b3[p, k*sz + t] = w3f[p, t, k]
            w3b3 = wp.tile([128, 3 * sz], dt.bfloat16, tag="w3b3")
            nc.vector.tensor_copy(
                _v(w3b3[:], [w3b3[:].ap[0], [1, sz], [sz, 3]]),
                w3f[:],
            )
            # k-major fp16 neighbor indices
            j3h3 = wp.tile([128, 3 * sz], dt.float16, tag="j3h3")
            nc.vector.tensor_copy(
                _v(j3h3[:], [j3h3[:].ap[0], [1, sz], [sz, 3]]),
                miall[:, tsl, 0:3],
            )
            wtiles[(b, gi)] = (w3b3, j3h3)

        def do_coarse(b):
            vall = sel.tile([128, NCH, 8], dt.float32, tag="vall")
            miall = sel.tile([128, NCH, 8], dt.uint16, tag="miall")
            valls.append(vall)
            mialls.append(miall)
            lhs_all, rhs_all = lhs_alls[b], rhs_alls[b]
            for t in range(NCH):
                psd = ps_d.tile([128, 1024], dt.float32, tag="psd")
                for hm in range(2):
                    nc.tensor.matmul(
                        psd[:, 512 * hm:512 * hm + 512],
                        lhs_all[:, 128 * t:128 * t + 128],
                        rhs_all[:, 512 * hm:512 * hm + 512],
                        start=True,
                        stop=True,
                    )
                nc.vector.max(out=vall[:, t, :], in_=psd[:])
                nc.vector.max_index(
                    out=miall[:, t, :], in_max=vall[:, t, :], in_values=psd[:]
                )
                for gi, (c0, sz) in enumerate(GROUPS[b]):
                    if t == c0 + sz - 1:
                        do_weights(b, gi, c0, sz)

        def do_gstage(b):
            # G = W0a^T @ KF -> G^T rows (bf16) staged to HBM
            kf16 = kfp.tile([128, 4, M], dt.bfloat16, tag="kf16")
            for cj in range(4):
                kf32 = kf32p.tile([128, M], dt.float32, tag="kf32")
                nc.sync.dma_start(kf32[:], kf_h.ap()[b, 128 * cj:128 * cj + 128, :])
                nc.scalar.copy(kf16[:, cj, :], kf32[:])
            gtsb = gtp.tile([128, MCH, D0], dt.bfloat16, tag="gtsb")
            for mt in range(MCH):
                pg = ps_mm.tile([128, 512], dt.float32, tag="mm")
                for cj in range(4):
                    nc.tensor.matmul(
                        pg[:, 0:D0],
                        kf16[:, cj, 128 * mt:128 * mt + 128],
                        w0_sb[:, cj, :],
                        start=(cj == 0),
                        stop=(cj == 3),
                    )
                nc.scalar.copy(gtsb[:, mt, :], pg[:, 0:D0])
            nc.sync.dma_start(
                _v(gt_h[b].ap(), [[D0, 128], [128 * D0, MCH], [1, D0]]),
                gtsb[:],
            )

        def do_group(b, gi, mul_on_dve):
            c0, sz = GROUPS[b][gi]
            qn = sz * 128
            big = sz == QCH
            tailb = b == nb - 1 and gi >= len(GROUPS[b]) - 2
            wp = wts if big else wts4
            gatp = gat if big else gat4
            gwpp = gwp if big else gwp4
            wbpp = wbp if big else wbp4
            mpp = mlpp if big else mlpp4
            w3b3, j3h3 = wtiles[(b, gi)]
            # --- weight broadcast row via HBM round-trip
            pswt = ps_trb.tile([48, 128], dt.bfloat16, tag="trb")
            nc.tensor.transpose(pswt[:3 * sz, :], w3b3[:], ident_b[:])
            wsb3 = wp.tile([3 * sz, 128], dt.bfloat16, tag="wsb3")
            (nc.vector.tensor_copy if tailb else nc.scalar.copy)(wsb3[:], pswt[:3 * sz, :])
            nc.sync.dma_start(
                _v(wr_h[b][gi].ap(), [[128, 3 * sz], [1, 128]]),
                wsb3[:],
            )
            wb3 = wbpp.tile([128, 3 * qn], dt.bfloat16, tag="wb3")

            # --- idxw3: wrap-16 layout of the 3 neighbor index lists
            psj = ps_tru.tile([48, 128], dt.float16, tag="trj")
            nc.tensor.transpose(psj[:3 * sz, :], j3h3[:], ident_u[:])
            mit3 = wp.tile([3 * sz, 128], dt.float16, tag="mit3")
            (nc.vector.tensor_copy if tailb else nc.scalar.copy)(mit3[:], psj[:3 * sz, :])
            idxw3 = wp.tile([128, 3 * qn // 16], dt.int16, tag="idxw3")
            for s in range(8):
                pst2 = ps_tru.tile([48, 128], dt.float16, tag="trj")
                nc.tensor.transpose(
                    pst2[:16, :3 * sz], mit3[:, 16 * s:16 * s + 16],
                    ident_u[:3 * sz, :3 * sz]
                )
                # pst2[p16, k*sz+t] -> idxw3[p16, k*(qn//16) + t*8 + s]
                (nc.vector.tensor_copy if tailb else nc.scalar.copy)(
                    _v(idxw3[:16, :],
                       [idxw3[:16, :].ap[0], [qn // 16, 3], [8, sz]],
                       off=s),
                    _v(pst2[:16, :3 * sz],
                       [pst2[:16, :3 * sz].ap[0], [sz, 3], [1, sz]]),
                )
            for gsz in (16, 32, 64):
                nc.gpsimd.dma_start(idxw3[gsz:2 * gsz, :], idxw3[0:gsz, :])
            nc.scalar.dma_start(
                wb3[:], AP(wr_h[b][gi], 0, [[0, 128], [1, 3 * qn]])
            )

            # --- one gather for all 3 neighbors (channel-major bf16)
            g3 = gatp.tile([128, 2, 3 * qn], dt.bfloat16, tag="g3")
            nc.gpsimd.dma_gather(
                g3[:],
                gt_h[b].ap(),
                idxw3[:],
                3 * qn,
                3 * qn,
                D0,
                transpose=True,
                single_packet=False,
            )
            # --- weight multiply (Pool; last group splits with idle DVE)
            gw3 = gwpp.tile([128, 2, 3 * qn], dt.bfloat16, tag="gw3")
            wbb = _v(wb3[:], [wb3[:].ap[0], [0, 2], [1, 3 * qn]])
            if mul_on_dve:
                nc.gpsimd.tensor_mul(gw3[:, 0, :], g3[:, 0, :], wb3[:])
                nc.vector.tensor_mul(gw3[:, 1, :], g3[:, 1, :], wb3[:])
            else:
                nc.gpsimd.tensor_mul(gw3[:], g3[:], wbb)

            # --- unknow_feats -> bf16
            uf16 = mpp.tile([128, 2, qn], dt.bfloat16, tag="uf16")
            for cj in range(2):
                uf32 = mpp.tile([128, qn], dt.float32, tag="uf32")
                nc.sync.dma_start(
                    uf32[:],
                    uf_h.ap()[b, 128 * cj:128 * cj + 128, 128 * c0:128 * c0 + qn],
                )
                (nc.gpsimd.tensor_copy if tailb else nc.scalar.copy)(uf16[:, cj, :], uf32[:])

            # --- MLP1: relu(W0b^T uf + sum_k inject(gw3_k)) -> h_t bf16
            h_t = mpp.tile([128, 2, qn], dt.bfloat16, tag="h")
            for mj in range(2):
                for nci in range(qn // 512):
                    nsl_ = slice(512 * nci, 512 * nci + 512)
                    pm = ps_mm.tile([128, 512], dt.float32, tag="mm")
                    for ci in range(2):
                        nc.tensor.matmul(
                            pm[:],
                            w0_sb[:, 4 + ci, 128 * mj:128 * mj + 128],
                            uf16[:, ci, nsl_],
                            start=(ci == 0),
                            stop=False,
                        )
                    for k in range(3):
                        nc.tensor.matmul(
                            pm[:],
                            ident_b[:],
                            gw3[:, mj, k * qn + 512 * nci:k * qn + 512 * nci + 512],
                            start=False,
                            stop=(k == 2),
                        )
                    if tailb and mj == 0:
                        nc.vector.tensor_scalar_max(h_t[:, mj, nsl_], pm[:], 0.0)
                    else:
                        nc.scalar.activation(h_t[:, mj, nsl_], pm[:], ACTF.Relu, bias=0.0)

            # --- MLP2 (relu) -> fp32 out
            for mj in range(2):
                o_t = mpp.tile([128, qn], dt.float32, tag="o")
                for nci in range(qn // 512):
                    nsl_ = slice(512 * nci, 512 * nci + 512)
                    pm = ps_mm.tile([128, 512], dt.float32, tag="mm")
                    for ci in range(2):
                        nc.tensor.matmul(
                            pm[:],
                            w1_sb[:, ci, 128 * mj:128 * mj + 128],
                            h_t[:, ci, nsl_],
                            start=(ci == 0),
                            stop=(ci == 1),
                        )
                    if tailb and mj == 0:
                        nc.vector.tensor_scalar_max(o_t[:, nsl_], pm[:], 0.0)
                    else:
                        nc.scalar.activation(o_t[:, nsl_], pm[:], ACTF.Relu, bias=0.0)
                nc.sync.dma_start(
                    out_h.ap()[b, 128 * mj:128 * mj + 128, 128 * c0:128 * c0 + qn],
                    o_t[:],
                )

        # ---- phase schedule: keep the DVE scan stream dense; batch 0's
        # gather/MLP work executes under batch 1's scans.
        do_prep(0)
        do_coarse(0)
        do_gstage(0)
        do_prep(1)
        do_coarse(1)
        do_gstage(1)
        for b in range(nb):
            for gi in range(len(GROUPS[b])):
                last = (b == nb - 1 and gi >= len(GROUPS[b]) - 2)
                do_group(b, gi, mul_on_dve=last)

    nc.compile()
    return nc


_NC_CACHE = {}


def _get_nc(nb=NB):
    if nb not in _NC_CACHE:
        _NC_CACHE[nb] = build_nc(nb)
    return _NC_CACHE[nb]


def kernel(**inputs):
    from concourse.bass_utils import run_bass_kernel_spmd

    nc = _get_nc(NB)
    per_core = B_FULL // N_CORES
    in_maps = []
    for c in range(N_CORES):
        sl = slice(per_core * c, per_core * (c + 1))
        in_maps.append(
            {
                "unknown": np.ascontiguousarray(np.asarray(inputs["unknown"][sl], dtype=np.float32)),
                "known": np.ascontiguousarray(np.asarray(inputs["known"][sl], dtype=np.float32)),
                "unknow_feats": np.ascontiguousarray(np.asarray(inputs["unknow_feats"][sl], dtype=np.float32)),
                "known_feats": np.ascontiguousarray(np.asarray(inputs["known_feats"][sl], dtype=np.float32)),
                "W0": np.asarray(inputs["W0"], dtype=np.float32),
                "W1": np.asarray(inputs["W1"], dtype=np.float32),
            }
        )
    res = run_bass_kernel_spmd(nc, in_maps, core_ids=list(range(N_CORES)))
    out = np.concatenate([res.results[c]["out"] for c in range(N_CORES)], axis=0)
    return out.astype(np.float32)


# revision 38
# speedup vs baseline: 1.0254x; 1.0254x over previous
# PointNet++ feature-propagation (three_nn + three_interpolate + shared MLP)
# Trainium2 Bass/Tile kernel, 8 NeuronCores, data-parallel over batch.
#
# Per batch (n=4096 unknown, m=1024 known, C2=512, C1=256):
#  1) ONE bf16 matmul (K=24 rows of triple-bf16 splits) computes
#     D' = 2u.k - |k|^2 - |u|^2 = -d2 directly in PSUM (error ~2e-6 abs);
#     Max/MaxIndex scan PSUM directly -> top-3 neighbors + their -d2.
#  2) weights from -d2 (clamped), no refine pass needed.
#  3) G = W0a^T @ known_feats is precomputed per batch ([256, m]); its
#     transpose rows (bf16, 512B) are staged to HBM and gathered per point
#     (3 neighbors in one DGE gather), halving gather bytes vs raw feats.
#  4) interp contribution = sum_k w_k * g_k is injected into the MLP1 PSUM
#     via identity matmuls; MLP1 = relu(W0b^T uf + inject), MLP2 as usual.
#
# The two batches are software-pipelined and the post-scan work is cut into
# quarter-batch (1024-point) groups so the DVE (which owns the unavoidable
# Max/MaxIndex scans, ~150us) never stalls and the post-scan tail is short.
import numpy as np
from contextlib import ExitStack

import concourse.bass as bass
import concourse.bacc as bacc
import concourse.tile as tile
import concourse.mybir as mybir
from concourse.masks import make_identity

AP = bass.AP
dt = mybir.dt
Alu = mybir.AluOpType
ACTF = mybir.ActivationFunctionType

B_FULL = 16
N_CORES = 8
NB = 2            # batches per core
N = 4096
M = 1024
C1 = 256
C2 = 512
D0 = 256
D1 = 256

NCH = N // 128    # 32 i-chunks
MCH = M // 128    # 8 j-chunks
QCH = 8           # i-chunks per group (quarter batch)
QN = QCH * 128    # 1024 points per group
NG = NCH // QCH   # 4 groups per batch
KROWS = 24
D2_FLOOR = 1e-7   # clamp for d2 (reference adds 1e-8; matmul err ~2e-6)
IDX_REPLICATE = True  # replicate idx wrap-table to 128 partitions (HW DGE req?)


def _bf16_split3(ve, pool, x_ap, shape, tagp):
    """bf16 (hi, lo, mid) with hi+lo+mid ~= x."""
    xh = pool.tile(list(shape), dt.bfloat16, tag=tagp + "_h")
    xl = pool.tile(list(shape), dt.bfloat16, tag=tagp + "_l")
    xm = pool.tile(list(shape), dt.bfloat16, tag=tagp + "_m")
    r1 = pool.tile(list(shape), dt.float32, tag=tagp + "_r1")
    r2 = pool.tile(list(shape), dt.float32, tag=tagp + "_r2")
    ve.tensor_copy(xh[:], x_ap)
    ve.tensor_sub(r1[:], x_ap, xh[:])
    ve.tensor_copy(xl[:], r1[:])
    ve.tensor_sub(r2[:], r1[:], xl[:])
    ve.tensor_copy(xm[:], r2[:])
    return xh, xl, xm


def _v(t_ap, dims, off=0):
    """AP over t_ap's tensor with explicit [stride, count] dims (dims[0] = partition dim)."""
    return AP(t_ap.tensor, t_ap.offset + off, dims)


def build_nc(nb=NB):
    nc = bacc.Bacc("TRN2", target_bir_lowering=False, debug=False)

    unknown_h = nc.dram_tensor("unknown", [nb, N, 3], dt.float32, kind="ExternalInput")
    known_h = nc.dram_tensor("known", [nb, M, 3], dt.float32, kind="ExternalInput")
    uf_h = nc.dram_tensor("unknow_feats", [nb, C1, N], dt.float32, kind="ExternalInput")
    kf_h = nc.dram_tensor("known_feats", [nb, C2, M], dt.float32, kind="ExternalInput")
    w0_h = nc.dram_tensor("W0", [C1 + C2, D0], dt.float32, kind="ExternalInput")
    w1_h = nc.dram_tensor("W1", [D0, D1], dt.float32, kind="ExternalInput")
    out_h = nc.dram_tensor("out", [nb, D1, N], dt.float32, kind="ExternalOutput")

    GROUPS = [[(0, 8), (8, 8), (16, 8), (24, 8)] for _ in range(nb)]
    GROUPS[nb - 1] = [(0, 8), (8, 8), (16, 8), (24, 4), (28, 4)]

    gt_h = [nc.dram_tensor(f"gt{b}", [M, D0], dt.bfloat16) for b in range(nb)]
    wr_h = [[nc.dram_tensor(f"wr{b}_{gi}", [3 * sz * 128], dt.bfloat16)
             for gi, (c0, sz) in enumerate(GROUPS[b])]
            for b in range(nb)]

    with tile.TileContext(nc) as tc, ExitStack() as ctx:
        const = ctx.enter_context(tc.tile_pool(name="const", bufs=1))
        kfp = ctx.enter_context(tc.tile_pool(name="kfp", bufs=2))
        kf32p = ctx.enter_context(tc.tile_pool(name="kf32p", bufs=1))
        gtp = ctx.enter_context(tc.tile_pool(name="gtp", bufs=1))
        prep = ctx.enter_context(tc.tile_pool(name="prep", bufs=2))
        sp = ctx.enter_context(tc.tile_pool(name="split", bufs=2))
        sel = ctx.enter_context(tc.tile_pool(name="sel", bufs=2))
        wts = ctx.enter_context(tc.tile_pool(name="wts", bufs=4))
        wts4 = ctx.enter_context(tc.tile_pool(name="wts4", bufs=2))
        gat = ctx.enter_context(tc.tile_pool(name="gat", bufs=2))
        gat4 = ctx.enter_context(tc.tile_pool(name="gat4", bufs=2))
        gwp = ctx.enter_context(tc.tile_pool(name="gwp", bufs=2))
        gwp4 = ctx.enter_context(tc.tile_pool(name="gwp4", bufs=2))
        wbp = ctx.enter_context(tc.tile_pool(name="wbp", bufs=2))
        wbp4 = ctx.enter_context(tc.tile_pool(name="wbp4", bufs=2))
        mlpp = ctx.enter_context(tc.tile_pool(name="mlpp", bufs=2))
        mlpp4 = ctx.enter_context(tc.tile_pool(name="mlpp4", bufs=2))
        ps_d = ctx.enter_context(tc.tile_pool(name="ps_d", bufs=2, space="PSUM"))
        ps_mm = ctx.enter_context(tc.tile_pool(name="ps_mm", bufs=2, space="PSUM"))
        ps_trb = ctx.enter_context(tc.tile_pool(name="ps_trb", bufs=1, space="PSUM"))
        ps_tru = ctx.enter_context(tc.tile_pool(name="ps_tru", bufs=1, space="PSUM"))

        # ---------------- constants ----------------
        ident_b = const.tile([128, 128], dt.bfloat16, tag="idb")
        make_identity(nc, ident_b[:])
        ident_u = const.tile([128, 128], dt.float16, tag="idu")
        make_identity(nc, ident_u[:])

        w0_sb = const.tile([128, 6, D0], dt.bfloat16, tag="w0")
        w1_sb = const.tile([128, 2, D1], dt.bfloat16, tag="w1")
        for ci in range(6):
            wtmp = kf32p.tile([128, D0], dt.float32, tag="wstg")
            nc.sync.dma_start(wtmp[:], w0_h.ap()[128 * ci:128 * ci + 128, :])
            nc.scalar.copy(w0_sb[:, ci, :], wtmp[:])
        for ci in range(2):
            wtmp = kf32p.tile([128, D1], dt.float32, tag="wstg")
            nc.sync.dma_start(wtmp[:], w1_h.ap()[128 * ci:128 * ci + 128, :])
            nc.scalar.copy(w1_sb[:, ci, :], wtmp[:])

        lhs_alls, rhs_alls = [], []
        valls, mialls = [], []
        wtiles = {}

        def do_prep(b):
            ve = nc.gpsimd
            cpe = nc.vector.tensor_copy if b == 0 else nc.scalar.copy
            # ---- known prep -> rhs_all
            kw = prep.tile([128, MCH, 3], dt.float32, tag="kw")
            nc.sync.dma_start(
                kw[:], AP(known_h, b * M * 3, [[3, 128], [3 * 128, MCH], [1, 3]])
            )
            k2 = prep.tile([128, MCH, 3], dt.float32, tag="k2")
            ve.tensor_scalar_mul(k2[:], kw[:], 2.0)
            k2h, k2l, k2m = _bf16_split3(ve, sp, k2[:], [128, MCH, 3], "k2")
            sq = prep.tile([128, MCH, 3], dt.float32, tag="ksq")
            nc.scalar.square(sq[:], kw[:])
            s_f = prep.tile([128, MCH], dt.float32, tag="ks")
            ve.tensor_add(s_f[:], sq[:, :, 0], sq[:, :, 1])
            ve.tensor_add(s_f[:], s_f[:], sq[:, :, 2])
            ns = prep.tile([128, MCH], dt.float32, tag="kns")
            ve.tensor_scalar_mul(ns[:], s_f[:], -1.0)
            nsh, nsl, nsm = _bf16_split3(ve, sp, ns[:], [128, MCH], "kns")

            # rows: 0-2 uh|2kh, 3-5 uh|2kl, 6-8 ul|2kh, 9-11 ul|2kl,
            #       12-14 uh|2km, 15-17 um|2kh, 18-20 1|-(skh,skl,skm),
            #       21-23 -(suh,sul,sum)|1
            kch = prep.tile([128, MCH, 32], dt.bfloat16, tag="kch")
            for (r0, src) in ((0, k2h), (3, k2l), (6, k2h), (9, k2l), (12, k2m), (15, k2h)):
                cpe(kch[:, :, r0:r0 + 3], src[:])
            cpe(kch[:, :, 18], nsh[:])
            cpe(kch[:, :, 19], nsl[:])
            cpe(kch[:, :, 20], nsm[:])
            ve.memset(kch[:, :, 21:24], 1.0)
            rhs_all = prep.tile([KROWS, M], dt.bfloat16, tag="rhs_all")
            for t in range(MCH):
                pst = ps_trb.tile([48, 128], dt.bfloat16, tag="trb")
                nc.tensor.transpose(pst[:KROWS, :], kch[:, t, :KROWS], ident_b[:])
                cpe(rhs_all[:, 128 * t:128 * t + 128], pst[:KROWS, :])

            # ---- unknown prep -> lhs_all
            uw = prep.tile([128, NCH, 3], dt.float32, tag="uw")
            nc.sync.dma_start(
                uw[:], AP(unknown_h, b * N * 3, [[3, 128], [3 * 128, NCH], [1, 3]])
            )
            uh, ul, um = _bf16_split3(ve, sp, uw[:], [128, NCH, 3], "u")
            usq = prep.tile([128, NCH, 3], dt.float32, tag="usq")
            nc.scalar.square(usq[:], uw[:])
            su = prep.tile([128, NCH], dt.float32, tag="us")
            ve.tensor_add(su[:], usq[:, :, 0], usq[:, :, 1])
            ve.tensor_add(su[:], su[:], usq[:, :, 2])
            nsu = prep.tile([128, NCH], dt.float32, tag="uns")
            ve.tensor_scalar_mul(nsu[:], su[:], -1.0)
            nsuh, nsul, nsum_ = _bf16_split3(ve, sp, nsu[:], [128, NCH], "uns")

            uch = prep.tile([128, NCH, 32], dt.bfloat16, tag="uch")
            for (r0, src) in ((0, uh), (3, uh), (6, ul), (9, ul), (12, uh), (15, um)):
                cpe(uch[:, :, r0:r0 + 3], src[:])
            ve.memset(uch[:, :, 18:21], 1.0)
            cpe(uch[:, :, 21], nsuh[:])
            cpe(uch[:, :, 22], nsul[:])
            cpe(uch[:, :, 23], nsum_[:])
            lhs_all = prep.tile([KROWS, N], dt.bfloat16, tag="lhs_all")
            for t in range(NCH):
                pst = ps_trb.tile([48, 128], dt.bfloat16, tag="trb")
                nc.tensor.transpose(pst[:KROWS, :], uch[:, t, :KROWS], ident_b[:])
                (cpe if t == 0 else nc.scalar.copy)(
                    lhs_all[:, 128 * t:128 * t + 128], pst[:KROWS, :])
            lhs_alls.append(lhs_all)
            rhs_alls.append(rhs_all)

        def do_weights(b, gi, c0, sz):
            # small DVE ops right after this group's scans: weights + idx list
            vall, miall = valls[b], mialls[b]
            wp = wts if sz == QCH else wts4
            tsl = slice(c0, c0 + sz)
            d23 = wp.tile([128, sz, 3], dt.float32, tag="d23")
            nc.vector.tensor_scalar(
                d23[:], vall[:, tsl, 0:3], -1.0, D2_FLOOR, op0=Alu.mult, op1=Alu.max
            )
            r3 = wp.tile([128, sz, 3], dt.float32, tag="r3")
            nc.vector.reciprocal(r3[:], d23[:])
            z = wp.tile([128, sz], dt.float32, tag="z")
            nc.vector.tensor_reduce(z[:], r3[:], axis=mybir.AxisListType.X, op=Alu.add)
            iz = wp.tile([128, sz], dt.float32, tag="iz")
            nc.vector.reciprocal(iz[:], z[:])
            w3f = wp.tile([128, sz, 3], dt.float32, tag="w3f")
            nc.vector.tensor_mul(w3f[:], r3[:], iz[:].to_broadcast([128, sz, 3]))
            # k-major bf16: w3b3[p, k*sz + t] = w3f[p, t, k]
            w3b3 = wp.tile([128, 3 * sz], dt.bfloat16, tag="w3b3")
            nc.vector.tensor_copy(
                _v(w3b3[:], [w3b3[:].ap[0], [1, sz], [sz, 3]]),
                w3f[:],
            )
            # k-major fp16 neighbor indices
            j3h3 = wp.tile([128, 3 * sz], dt.float16, tag="j3h3")
            nc.vector.tensor_copy(
                _v(j3h3[:], [j3h3[:].ap[0], [1, sz], [sz, 3]]),
                miall[:, tsl, 0:3],
            )
            wtiles[(b, gi)] = (w3b3, j3h3)

        def do_coarse(b):
            vall = sel.tile([128, NCH, 8], dt.float32, tag="vall")
            miall = sel.tile([128, NCH, 8], dt.uint16, tag="miall")
            valls.append(vall)
            mialls.append(miall)
            lhs_all, rhs_all = lhs_alls[b], rhs_alls[b]
            for t in range(NCH):
                psd = ps_d.tile([128, 1024], dt.float32, tag="psd")
                for hm in range(2):
                    nc.tensor.matmul(
                        psd[:, 512 * hm:512 * hm + 512],
                        lhs_all[:, 128 * t:128 * t + 128],
                        rhs_all[:, 512 * hm:512 * hm + 512],
                        start=True,
                        stop=True,
                    )
                nc.vector.max(out=vall[:, t, :], in_=psd[:])
                nc.vector.max_index(
                    out=miall[:, t, :], in_max=vall[:, t, :], in_values=psd[:]
                )
                for gi, (c0, sz) in enumerate(GROUPS[b]):
                    if t == c0 + sz - 1:
                        do_weights(b, gi, c0, sz)

        def do_gstage(b):
            # G = W0a^T @ KF -> G^T rows (bf16) staged to HBM
            kf16 = kfp.tile([128, 4, M], dt.bfloat16, tag="kf16")
            for cj in range(4):
                kf32 = kf32p.tile([128, M], dt.float32, tag="kf32")
                nc.sync.dma_start(kf32[:], kf_h.ap()[b, 128 * cj:128 * cj + 128, :])
                nc.scalar.copy(kf16[:, cj, :], kf32[:])
            gtsb = gtp.tile([128, MCH, D0], dt.bfloat16, tag="gtsb")
            for mt in range(MCH):
                pg = ps_mm.tile([128, 512], dt.float32, tag="mm")
                for cj in range(4):
                    nc.tensor.matmul(
                        pg[:, 0:D0],
                        kf16[:, cj, 128 * mt:128 * mt + 128],
                        w0_sb[:, cj, :],
                        start=(cj == 0),
                        stop=(cj == 3),
                    )
                nc.scalar.copy(gtsb[:, mt, :], pg[:, 0:D0])
            nc.sync.dma_start(
                _v(gt_h[b].ap(), [[D0, 128], [128 * D0, MCH], [1, D0]]),
                gtsb[:],
            )

        def do_group(b, gi, mul_on_dve):
            c0, sz = GROUPS[b][gi]
            qn = sz * 128
            big = sz == QCH
            tailb = b == nb - 1 and gi >= len(GROUPS[b]) - 2
            wp = wts if big else wts4
            gatp = gat if big else gat4
            gwpp = gwp if big else gwp4
            wbpp = wbp if big else wbp4
            mpp = mlpp if big else mlpp4
            w3b3, j3h3 = wtiles[(b, gi)]
            # --- weight broadcast row via HBM round-trip
            pswt = ps_trb.tile([48, 128], dt.bfloat16, tag="trb")
            nc.tensor.transpose(pswt[:3 * sz, :], w3b3[:], ident_b[:])
            wsb3 = wp.tile([3 * sz, 128], dt.bfloat16, tag="wsb3")
            (nc.vector.tensor_copy if tailb else nc.scalar.copy)(wsb3[:], pswt[:3 * sz, :])
            nc.sync.dma_start(
                _v(wr_h[b][gi].ap(), [[128, 3 * sz], [1, 128]]),
                wsb3[:],
            )
            wb3 = wbpp.tile([128, 3 * qn], dt.bfloat16, tag="wb3")

            # --- idxw3: wrap-16 layout of the 3 neighbor index lists
            psj = ps_tru.tile([48, 128], dt.float16, tag="trj")
            nc.tensor.transpose(psj[:3 * sz, :], j3h3[:], ident_u[:])
            mit3 = wp.tile([3 * sz, 128], dt.float16, tag="mit3")
            (nc.vector.tensor_copy if tailb else nc.scalar.copy)(mit3[:], psj[:3 * sz, :])
            idxw3 = wp.tile([128, 3 * qn // 16], dt.int16, tag="idxw3")
            for s in range(8):
                pst2 = ps_tru.tile([48, 128], dt.float16, tag="trj")
                nc.tensor.transpose(
                    pst2[:16, :3 * sz], mit3[:, 16 * s:16 * s + 16],
                    ident_u[:3 * sz, :3 * sz]
                )
                # pst2[p16, k*sz+t] -> idxw3[p16, k*(qn//16) + t*8 + s]
                (nc.vector.tensor_copy if tailb else nc.scalar.copy)(
                    _v(idxw3[:16, :],
                       [idxw3[:16, :].ap[0], [qn // 16, 3], [8, sz]],
                       off=s),
                    _v(pst2[:16, :3 * sz],
                       [pst2[:16, :3 * sz].ap[0], [sz, 3], [1, sz]]),
                )
            if IDX_REPLICATE:
                for r in range(1, 8):
                    if tailb:
                        eng = nc.sync if r % 2 == 0 else nc.scalar
                    else:
                        eng = nc.gpsimd
                    eng.dma_start(idxw3[16 * r:16 * r + 16, :], idxw3[0:16, :])
            nc.scalar.dma_start(
                wb3[:], AP(wr_h[b][gi], 0, [[0, 128], [1, 3 * qn]])
            )

            # --- one gather for all 3 neighbors (channel-major bf16)
            g3 = gatp.tile([128, 2, 3 * qn], dt.bfloat16, tag="g3")
            nc.gpsimd.dma_gather(
                g3[:],
                gt_h[b].ap(),
                idxw3[:],
                3 * qn,
                3 * qn,
                D0,
                transpose=True,
                single_packet=False,
            )
            # --- weight multiply (Pool; last group splits with idle DVE)
            gw3 = gwpp.tile([128, 2, 3 * qn], dt.bfloat16, tag="gw3")
            wbb = _v(wb3[:], [wb3[:].ap[0], [0, 2], [1, 3 * qn]])
            if mul_on_dve:
                nc.gpsimd.tensor_mul(gw3[:, 0, :], g3[:, 0, :], wb3[:])
                nc.vector.tensor_mul(gw3[:, 1, :], g3[:, 1, :], wb3[:])
            else:
                nc.gpsimd.tensor_mul(gw3[:], g3[:], wbb)

            # --- unknow_feats -> bf16
            uf16 = mpp.tile([128, 2, qn], dt.bfloat16, tag="uf16")
            for cj in range(2):
                uf32 = mpp.tile([128, qn], dt.float32, tag="uf32")
                nc.sync.dma_start(
                    uf32[:],
                    uf_h.ap()[b, 128 * cj:128 * cj + 128, 128 * c0:128 * c0 + qn],
                )
                (nc.gpsimd.tensor_copy if tailb else nc.scalar.copy)(uf16[:, cj, :], uf32[:])

            # --- MLP1: relu(W0b^T uf + sum_k inject(gw3_k)) -> h_t bf16
            h_t = mpp.tile([128, 2, qn], dt.bfloat16, tag="h")
            for mj in range(2):
                for nci in range(qn // 512):
                    nsl_ = slice(512 * nci, 512 * nci + 512)
                    pm = ps_mm.tile([128, 512], dt.float32, tag="mm")
                    for ci in range(2):
                        nc.tensor.matmul(
                            pm[:],
                            w0_sb[:, 4 + ci, 128 * mj:128 * mj + 128],
                            uf16[:, ci, nsl_],
                            start=(ci == 0),
                            stop=False,
                        )
                    for k in range(3):
                        nc.tensor.matmul(
                            pm[:],
                            ident_b[:],
                            gw3[:, mj, k * qn + 512 * nci:k * qn + 512 * nci + 512],
                            start=False,
                            stop=(k == 2),
                        )
                    if tailb and mj == 0:
                        nc.vector.tensor_scalar_max(h_t[:, mj, nsl_], pm[:], 0.0)
                    else:
                        nc.scalar.activation(h_t[:, mj, nsl_], pm[:], ACTF.Relu, bias=0.0)

            # --- MLP2 (relu) -> fp32 out
            for mj in range(2):
                o_t = mpp.tile([128, qn], dt.float32, tag="o")
                for nci in range(qn // 512):
                    nsl_ = slice(512 * nci, 512 * nci + 512)
                    pm = ps_mm.tile([128, 512], dt.float32, tag="mm")
                    for ci in range(2):
                        nc.tensor.matmul(
                            pm[:],
                            w1_sb[:, ci, 128 * mj:128 * mj + 128],
                            h_t[:, ci, nsl_],
                            start=(ci == 0),
                            stop=(ci == 1),
                        )
                    if tailb and mj == 0:
                        nc.vector.tensor_scalar_max(o_t[:, nsl_], pm[:], 0.0)
                    else:
                        nc.scalar.activation(o_t[:, nsl_], pm[:], ACTF.Relu, bias=0.0)
                nc.sync.dma_start(
                    out_h.ap()[b, 128 * mj:128 * mj + 128, 128 * c0:128 * c0 + qn],
                    o_t[:],
                )

        # ---- phase schedule: keep the DVE scan stream dense; batch 0's
        # gather/MLP work executes under batch 1's scans.
        do_prep(0)
        do_coarse(0)
        do_gstage(0)
        do_prep(1)
        do_coarse(1)
        do_gstage(1)
        for b in range(nb):
            for gi in range(len(GROUPS[b])):
                last = (b == nb - 1 and gi >= len(GROUPS[b]) - 2)
                do_group(b, gi, mul_on_dve=last)

    nc.compile()
    return nc


_NC_CACHE = {}


def _get_nc(nb=NB):
    if nb not in _NC_CACHE:
        _NC_CACHE[nb] = build_nc(nb)
    return _NC_CACHE[nb]


def kernel(**inputs):
    from concourse.bass_utils import run_bass_kernel_spmd

    nc = _get_nc(NB)
    per_core = B_FULL // N_CORES
    in_maps = []
    for c in range(N_CORES):
        sl = slice(per_core * c, per_core * (c + 1))
        in_maps.append(
            {
                "unknown": np.ascontiguousarray(np.asarray(inputs["unknown"][sl], dtype=np.float32)),
                "known": np.ascontiguousarray(np.asarray(inputs["known"][sl], dtype=np.float32)),
                "unknow_feats": np.ascontiguousarray(np.asarray(inputs["unknow_feats"][sl], dtype=np.float32)),
                "known_feats": np.ascontiguousarray(np.asarray(inputs["known_feats"][sl], dtype=np.float32)),
                "W0": np.asarray(inputs["W0"], dtype=np.float32),
                "W1": np.asarray(inputs["W1"], dtype=np.float32),
            }
        )
    res = run_bass_kernel_spmd(nc, in_maps, core_ids=list(range(N_CORES)))
    out = np.concatenate([res.results[c]["out"] for c in range(N_CORES)], axis=0)
    return out.astype(np.float32)


# revision 43
# speedup vs baseline: 1.0396x; 1.0139x over previous
# PointNet++ feature-propagation (three_nn + three_interpolate + shared MLP)
# Trainium2 Bass/Tile kernel, 8 NeuronCores, data-parallel over batch.
#
# Per batch (n=4096 unknown, m=1024 known, C2=512, C1=256):
#  1) ONE bf16 matmul (K=24 rows of triple-bf16 splits) computes
#     D' = 2u.k - |k|^2 - |u|^2 = -d2 directly in PSUM (error ~2e-6 abs);
#     Max/MaxIndex scan PSUM directly -> top-3 neighbors + their -d2.
#  2) weights from -d2 (clamped), no refine pass needed.
#  3) G = W0a^T @ known_feats is precomputed per batch ([256, m]); its
#     transpose rows (bf16, 512B) are staged to HBM and gathered per point
#     (3 neighbors in one DGE gather), halving gather bytes vs raw feats.
#  4) interp contribution = sum_k w_k * g_k is injected into the MLP1 PSUM
#     via identity matmuls; MLP1 = relu(W0b^T uf + inject), MLP2 as usual.
#
# The two batches are software-pipelined and the post-scan work is cut into
# quarter-batch (1024-point) groups so the DVE (which owns the unavoidable
# Max/MaxIndex scans, ~150us) never stalls and the post-scan tail is short.
import numpy as np
from contextlib import ExitStack

import concourse.bass as bass
import concourse.bacc as bacc
import concourse.tile as tile
import concourse.mybir as mybir
from concourse.masks import make_identity

AP = bass.AP
dt = mybir.dt
Alu = mybir.AluOpType
ACTF = mybir.ActivationFunctionType

B_FULL = 16
N_CORES = 8
NB = 2            # batches per core
N = 4096
M = 1024
C1 = 256
C2 = 512
D0 = 256
D1 = 256

NCH = N // 128    # 32 i-chunks
MCH = M // 128    # 8 j-chunks
QCH = 8           # i-chunks per group (quarter batch)
QN = QCH * 128    # 1024 points per group
NG = NCH // QCH   # 4 groups per batch
KROWS = 24
D2_FLOOR = 1e-7   # clamp for d2 (reference adds 1e-8; matmul err ~2e-6)
IDX_REPLICATE = True  # replicate idx wrap-table to 128 partitions (HW DGE req?)


def _bf16_split3(ve, pool, x_ap, shape, tagp):
    """bf16 (hi, lo, mid) with hi+lo+mid ~= x."""
    xh = pool.tile(list(shape), dt.bfloat16, tag=tagp + "_h")
    xl = pool.tile(list(shape), dt.bfloat16, tag=tagp + "_l")
    xm = pool.tile(list(shape), dt.bfloat16, tag=tagp + "_m")
    r1 = pool.tile(list(shape), dt.float32, tag=tagp + "_r1")
    r2 = pool.tile(list(shape), dt.float32, tag=tagp + "_r2")
    ve.tensor_copy(xh[:], x_ap)
    ve.tensor_sub(r1[:], x_ap, xh[:])
    ve.tensor_copy(xl[:], r1[:])
    ve.tensor_sub(r2[:], r1[:], xl[:])
    ve.tensor_copy(xm[:], r2[:])
    return xh, xl, xm


def _v(t_ap, dims, off=0):
    """AP over t_ap's tensor with explicit [stride, count] dims (dims[0] = partition dim)."""
    return AP(t_ap.tensor, t_ap.offset + off, dims)


def build_nc(nb=NB):
    nc = bacc.Bacc("TRN2", target_bir_lowering=False, debug=False)

    unknown_h = nc.dram_tensor("unknown", [nb, N, 3], dt.float32, kind="ExternalInput")
    known_h = nc.dram_tensor("known", [nb, M, 3], dt.float32, kind="ExternalInput")
    uf_h = nc.dram_tensor("unknow_feats", [nb, C1, N], dt.float32, kind="ExternalInput")
    kf_h = nc.dram_tensor("known_feats", [nb, C2, M], dt.float32, kind="ExternalInput")
    w0_h = nc.dram_tensor("W0", [C1 + C2, D0], dt.float32, kind="ExternalInput")
    w1_h = nc.dram_tensor("W1", [D0, D1], dt.float32, kind="ExternalInput")
    out_h = nc.dram_tensor("out", [nb, D1, N], dt.float32, kind="ExternalOutput")

    GROUPS = [[(0, 8), (8, 8), (16, 8), (24, 8)] for _ in range(nb)]
    GROUPS[nb - 1] = [(0, 8), (8, 8), (16, 8), (24, 4), (28, 4)]

    gt_h = [nc.dram_tensor(f"gt{b}", [M, D0], dt.bfloat16) for b in range(nb)]
    wr_h = [[nc.dram_tensor(f"wr{b}_{gi}", [3 * sz * 128], dt.bfloat16)
             for gi, (c0, sz) in enumerate(GROUPS[b])]
            for b in range(nb)]

    with tile.TileContext(nc) as tc, ExitStack() as ctx:
        const = ctx.enter_context(tc.tile_pool(name="const", bufs=1))
        kfp = ctx.enter_context(tc.tile_pool(name="kfp", bufs=2))
        kf32p = ctx.enter_context(tc.tile_pool(name="kf32p", bufs=1))
        gtp = ctx.enter_context(tc.tile_pool(name="gtp", bufs=1))
        prep = ctx.enter_context(tc.tile_pool(name="prep", bufs=2))
        sp = ctx.enter_context(tc.tile_pool(name="split", bufs=2))
        sel = ctx.enter_context(tc.tile_pool(name="sel", bufs=2))
        wts = ctx.enter_context(tc.tile_pool(name="wts", bufs=4))
        wts4 = ctx.enter_context(tc.tile_pool(name="wts4", bufs=2))
        gat = ctx.enter_context(tc.tile_pool(name="gat", bufs=2))
        gat4 = ctx.enter_context(tc.tile_pool(name="gat4", bufs=2))
        gwp = ctx.enter_context(tc.tile_pool(name="gwp", bufs=2))
        gwp4 = ctx.enter_context(tc.tile_pool(name="gwp4", bufs=2))
        wbp = ctx.enter_context(tc.tile_pool(name="wbp", bufs=2))
        wbp4 = ctx.enter_context(tc.tile_pool(name="wbp4", bufs=2))
        mlpp = ctx.enter_context(tc.tile_pool(name="mlpp", bufs=2))
        mlpp4 = ctx.enter_context(tc.tile_pool(name="mlpp4", bufs=2))
        ps_d = ctx.enter_context(tc.tile_pool(name="ps_d", bufs=2, space="PSUM"))
        ps_mm = ctx.enter_context(tc.tile_pool(name="ps_mm", bufs=2, space="PSUM"))
        ps_trb = ctx.enter_context(tc.tile_pool(name="ps_trb", bufs=1, space="PSUM"))
        ps_tru = ctx.enter_context(tc.tile_pool(name="ps_tru", bufs=1, space="PSUM"))

        # ---------------- constants ----------------
        ident_b = const.tile([128, 128], dt.bfloat16, tag="idb")
        make_identity(nc, ident_b[:])
        ident_u = const.tile([128, 128], dt.float16, tag="idu")
        make_identity(nc, ident_u[:])

        w0_sb = const.tile([128, 6, D0], dt.bfloat16, tag="w0")
        w1_sb = const.tile([128, 2, D1], dt.bfloat16, tag="w1")
        for ci in range(6):
            wtmp = kf32p.tile([128, D0], dt.float32, tag="wstg")
            nc.sync.dma_start(wtmp[:], w0_h.ap()[128 * ci:128 * ci + 128, :])
            nc.scalar.copy(w0_sb[:, ci, :], wtmp[:])
        for ci in range(2):
            wtmp = kf32p.tile([128, D1], dt.float32, tag="wstg")
            nc.sync.dma_start(wtmp[:], w1_h.ap()[128 * ci:128 * ci + 128, :])
            nc.scalar.copy(w1_sb[:, ci, :], wtmp[:])

        lhs_alls, rhs_alls = [], []
        valls, mialls = [], []
        wtiles = {}

        def do_prep(b):
            ve = nc.gpsimd
            cpe = nc.vector.tensor_copy if b == 0 else nc.scalar.copy
            # ---- known prep -> rhs_all
            kw = prep.tile([128, MCH, 3], dt.float32, tag="kw")
            nc.sync.dma_start(
                kw[:], AP(known_h, b * M * 3, [[3, 128], [3 * 128, MCH], [1, 3]])
            )
            k2 = prep.tile([128, MCH, 3], dt.float32, tag="k2")
            ve.tensor_scalar_mul(k2[:], kw[:], 2.0)
            k2h, k2l, k2m = _bf16_split3(ve, sp, k2[:], [128, MCH, 3], "k2")
            sq = prep.tile([128, MCH, 3], dt.float32, tag="ksq")
            nc.scalar.square(sq[:], kw[:])
            s_f = prep.tile([128, MCH], dt.float32, tag="ks")
            ve.tensor_add(s_f[:], sq[:, :, 0], sq[:, :, 1])
            ve.tensor_add(s_f[:], s_f[:], sq[:, :, 2])
            ns = prep.tile([128, MCH], dt.float32, tag="kns")
            ve.tensor_scalar_mul(ns[:], s_f[:], -1.0)
            nsh, nsl, nsm = _bf16_split3(ve, sp, ns[:], [128, MCH], "kns")

            # rows: 0-2 uh|2kh, 3-5 uh|2kl, 6-8 ul|2kh, 9-11 ul|2kl,
            #       12-14 uh|2km, 15-17 um|2kh, 18-20 1|-(skh,skl,skm),
            #       21-23 -(suh,sul,sum)|1
            kch = prep.tile([128, MCH, 32], dt.bfloat16, tag="kch")
            for (r0, src) in ((0, k2h), (3, k2l), (6, k2h), (9, k2l), (12, k2m), (15, k2h)):
                cpe(kch[:, :, r0:r0 + 3], src[:])
            cpe(kch[:, :, 18], nsh[:])
            cpe(kch[:, :, 19], nsl[:])
            cpe(kch[:, :, 20], nsm[:])
            ve.memset(kch[:, :, 21:24], 1.0)
            rhs_all = prep.tile([KROWS, M], dt.bfloat16, tag="rhs_all")
            for t in range(MCH):
                pst = ps_trb.tile([48, 128], dt.bfloat16, tag="trb")
                nc.tensor.transpose(pst[:KROWS, :], kch[:, t, :KROWS], ident_b[:])
                cpe(rhs_all[:, 128 * t:128 * t + 128], pst[:KROWS, :])

            # ---- unknown prep -> lhs_all
            uw = prep.tile([128, NCH, 3], dt.float32, tag="uw")
            nc.sync.dma_start(
                uw[:], AP(unknown_h, b * N * 3, [[3, 128], [3 * 128, NCH], [1, 3]])
            )
            uh, ul, um = _bf16_split3(ve, sp, uw[:], [128, NCH, 3], "u")
            usq = prep.tile([128, NCH, 3], dt.float32, tag="usq")
            nc.scalar.square(usq[:], uw[:])
            su = prep.tile([128, NCH], dt.float32, tag="us")
            ve.tensor_add(su[:], usq[:, :, 0], usq[:, :, 1])
            ve.tensor_add(su[:], su[:], usq[:, :, 2])
            nsu = prep.tile([128, NCH], dt.float32, tag="uns")
            ve.tensor_scalar_mul(nsu[:], su[:], -1.0)
            nsuh, nsul, nsum_ = _bf16_split3(ve, sp, nsu[:], [128, NCH], "uns")

            uch = prep.tile([128, NCH, 32], dt.bfloat16, tag="uch")
            for (r0, src) in ((0, uh), (3, uh), (6, ul), (9, ul), (12, uh), (15, um)):
                cpe(uch[:, :, r0:r0 + 3], src[:])
            ve.memset(uch[:, :, 18:21], 1.0)
            cpe(uch[:, :, 21], nsuh[:])
            cpe(uch[:, :, 22], nsul[:])
            cpe(uch[:, :, 23], nsum_[:])
            lhs_all = prep.tile([KROWS, N], dt.bfloat16, tag="lhs_all")
            for t in range(NCH):
                pst = ps_trb.tile([48, 128], dt.bfloat16, tag="trb")
                nc.tensor.transpose(pst[:KROWS, :], uch[:, t, :KROWS], ident_b[:])
                (cpe if t == 0 else nc.scalar.copy)(
                    lhs_all[:, 128 * t:128 * t + 128], pst[:KROWS, :])
            lhs_alls.append(lhs_all)
            rhs_alls.append(rhs_all)

        def do_weights(b, gi, c0, sz):
            # small DVE ops right after this group's scans: weights + idx list
            vall, miall = valls[b], mialls[b]
            wp = wts if sz == QCH else wts4
            tsl = slice(c0, c0 + sz)
            d23 = wp.tile([128, sz, 3], dt.float32, tag="d23")
            nc.vector.tensor_scalar(
                d23[:], vall[:, tsl, 0:3], -1.0, D2_FLOOR, op0=Alu.mult, op1=Alu.max
            )
            r3 = wp.tile([128, sz, 3], dt.float32, tag="r3")
            nc.vector.reciprocal(r3[:], d23[:])
            z = wp.tile([128, sz], dt.float32, tag="z")
            nc.vector.tensor_reduce(z[:], r3[:], axis=mybir.AxisListType.X, op=Alu.add)
            iz = wp.tile([128, sz], dt.float32, tag="iz")
            nc.vector.reciprocal(iz[:], z[:])
            w3f = wp.tile([128, sz, 3], dt.float32, tag="w3f")
            nc.vector.tensor_mul(w3f[:], r3[:], iz[:].to_broadcast([128, sz, 3]))
            # k-major bf16: w3b3[p, k*sz + t] = w3f[p, t, k]
            w3b3 = wp.tile([128, 3 * sz], dt.bfloat16, tag="w3b3")
            nc.vector.tensor_copy(
                _v(w3b3[:], [w3b3[:].ap[0], [1, sz], [sz, 3]]),
                w3f[:],
            )
            # k-major fp16 neighbor indices
            j3h3 = wp.tile([128, 3 * sz], dt.float16, tag="j3h3")
            nc.vector.tensor_copy(
                _v(j3h3[:], [j3h3[:].ap[0], [1, sz], [sz, 3]]),
                miall[:, tsl, 0:3],
            )
            wtiles[(b, gi)] = (w3b3, j3h3)

        def do_coarse(b):
            vall = sel.tile([128, NCH, 8], dt.float32, tag="vall")
            miall = sel.tile([128, NCH, 8], dt.uint16, tag="miall")
            valls.append(vall)
            mialls.append(miall)
            lhs_all, rhs_all = lhs_alls[b], rhs_alls[b]
            for t in range(NCH):
                psd = ps_d.tile([128, 1024], dt.float32, tag="psd")
                for hm in range(2):
                    nc.tensor.matmul(
                        psd[:, 512 * hm:512 * hm + 512],
                        lhs_all[:, 128 * t:128 * t + 128],
                        rhs_all[:, 512 * hm:512 * hm + 512],
                        start=True,
                        stop=True,
                    )
                nc.vector.max(out=vall[:, t, :], in_=psd[:])
                nc.vector.max_index(
                    out=miall[:, t, :], in_max=vall[:, t, :], in_values=psd[:]
                )
                for gi, (c0, sz) in enumerate(GROUPS[b]):
                    if t == c0 + sz - 1:
                        do_weights(b, gi, c0, sz)

        def do_gstage(b):
            # G = W0a^T @ KF -> G^T rows (bf16) staged to HBM
            kf16 = kfp.tile([128, 4, M], dt.bfloat16, tag="kf16")
            for cj in range(4):
                kf32 = kf32p.tile([128, M], dt.float32, tag="kf32")
                nc.sync.dma_start(kf32[:], kf_h.ap()[b, 128 * cj:128 * cj + 128, :])
                nc.scalar.copy(kf16[:, cj, :], kf32[:])
            gtsb = gtp.tile([128, MCH, D0], dt.bfloat16, tag="gtsb")
            for mt in range(MCH):
                pg = ps_mm.tile([128, 512], dt.float32, tag="mm")
                for cj in range(4):
                    nc.tensor.matmul(
                        pg[:, 0:D0],
                        kf16[:, cj, 128 * mt:128 * mt + 128],
                        w0_sb[:, cj, :],
                        start=(cj == 0),
                        stop=(cj == 3),
                    )
                nc.scalar.copy(gtsb[:, mt, :], pg[:, 0:D0])
            nc.sync.dma_start(
                _v(gt_h[b].ap(), [[D0, 128], [128 * D0, MCH], [1, D0]]),
                gtsb[:],
            )

        def do_group(b, gi, mul_on_dve):
            c0, sz = GROUPS[b][gi]
            qn = sz * 128
            big = sz == QCH
            tailb = b == nb - 1 and gi >= len(GROUPS[b]) - 2
            wp = wts if big else wts4
            gatp = gat if big else gat4
            gwpp = gwp if big else gwp4
            wbpp = wbp if big else wbp4
            mpp = mlpp if big else mlpp4
            w3b3, j3h3 = wtiles[(b, gi)]
            # --- weight broadcast row via HBM round-trip
            pswt = ps_trb.tile([48, 128], dt.bfloat16, tag="trb")
            nc.tensor.transpose(pswt[:3 * sz, :], w3b3[:], ident_b[:])
            wsb3 = wp.tile([3 * sz, 128], dt.bfloat16, tag="wsb3")
            nc.scalar.copy(wsb3[:], pswt[:3 * sz, :])
            nc.sync.dma_start(
                _v(wr_h[b][gi].ap(), [[128, 3 * sz], [1, 128]]),
                wsb3[:],
            )
            wb3 = wbpp.tile([128, 3 * qn], dt.bfloat16, tag="wb3")

            # --- idxw3: wrap-16 layout of the 3 neighbor index lists
            psj = ps_tru.tile([48, 128], dt.float16, tag="trj")
            nc.tensor.transpose(psj[:3 * sz, :], j3h3[:], ident_u[:])
            mit3 = wp.tile([3 * sz, 128], dt.float16, tag="mit3")
            (nc.vector.tensor_copy if tailb else nc.scalar.copy)(mit3[:], psj[:3 * sz, :])
            idxw3 = wp.tile([128, 3 * qn // 16], dt.int16, tag="idxw3")
            for s in range(8):
                pst2 = ps_tru.tile([48, 128], dt.float16, tag="trj")
                nc.tensor.transpose(
                    pst2[:16, :3 * sz], mit3[:, 16 * s:16 * s + 16],
                    ident_u[:3 * sz, :3 * sz]
                )
                # pst2[p16, k*sz+t] -> idxw3[p16, k*(qn//16) + t*8 + s]
                (nc.vector.tensor_copy if tailb else nc.scalar.copy)(
                    _v(idxw3[:16, :],
                       [idxw3[:16, :].ap[0], [qn // 16, 3], [8, sz]],
                       off=s),
                    _v(pst2[:16, :3 * sz],
                       [pst2[:16, :3 * sz].ap[0], [sz, 3], [1, sz]]),
                )
            if IDX_REPLICATE:
                for r in range(1, 8):
                    if tailb:
                        eng = nc.sync if r % 2 == 0 else nc.scalar
                    else:
                        eng = nc.gpsimd
                    eng.dma_start(idxw3[16 * r:16 * r + 16, :], idxw3[0:16, :])
            (nc.sync if tailb else nc.scalar).dma_start(
                wb3[:], AP(wr_h[b][gi], 0, [[0, 128], [1, 3 * qn]])
            )

            # --- one gather for all 3 neighbors (channel-major bf16)
            g3 = gatp.tile([128, 2, 3 * qn], dt.bfloat16, tag="g3")
            nc.gpsimd.dma_gather(
                g3[:],
                gt_h[b].ap(),
                idxw3[:],
                3 * qn,
                3 * qn,
                D0,
                transpose=True,
                single_packet=False,
            )
            # --- weight multiply (Pool; last group splits with idle DVE)
            gw3 = gwpp.tile([128, 2, 3 * qn], dt.bfloat16, tag="gw3")
            wbb = _v(wb3[:], [wb3[:].ap[0], [0, 2], [1, 3 * qn]])
            if mul_on_dve:
                nc.vector.tensor_mul(gw3[:, 0, :], g3[:, 0, :], wb3[:])
                nc.vector.tensor_mul(gw3[:, 1, :], g3[:, 1, :], wb3[:])
            else:
                nc.gpsimd.tensor_mul(gw3[:], g3[:], wbb)

            # --- unknow_feats -> bf16
            uf16 = mpp.tile([128, 2, qn], dt.bfloat16, tag="uf16")
            for cj in range(2):
                uf32 = mpp.tile([128, qn], dt.float32, tag="uf32")
                nc.sync.dma_start(
                    uf32[:],
                    uf_h.ap()[b, 128 * cj:128 * cj + 128, 128 * c0:128 * c0 + qn],
                )
                (nc.gpsimd.tensor_copy if tailb else nc.scalar.copy)(uf16[:, cj, :], uf32[:])

            # --- MLP1: relu(W0b^T uf + sum_k inject(gw3_k)) -> h_t bf16
            h_t = mpp.tile([128, 2, qn], dt.bfloat16, tag="h")
            for mj in range(2):
                for nci in range(qn // 512):
                    nsl_ = slice(512 * nci, 512 * nci + 512)
                    pm = ps_mm.tile([128, 512], dt.float32, tag="mm")
                    for ci in range(2):
                        nc.tensor.matmul(
                            pm[:],
                            w0_sb[:, 4 + ci, 128 * mj:128 * mj + 128],
                            uf16[:, ci, nsl_],
                            start=(ci == 0),
                            stop=False,
                        )
                    for k in range(3):
                        nc.tensor.matmul(
                            pm[:],
                            ident_b[:],
                            gw3[:, mj, k * qn + 512 * nci:k * qn + 512 * nci + 512],
                            start=False,
                            stop=(k == 2),
                        )
                    if tailb and mj == 0:
                        nc.vector.tensor_scalar_max(h_t[:, mj, nsl_], pm[:], 0.0)
                    else:
                        nc.scalar.activation(h_t[:, mj, nsl_], pm[:], ACTF.Relu, bias=0.0)

            # --- MLP2 (relu) -> fp32 out
            for mj in range(2):
                o_t = mpp.tile([128, qn], dt.float32, tag="o")
                for nci in range(qn // 512):
                    nsl_ = slice(512 * nci, 512 * nci + 512)
                    pm = ps_mm.tile([128, 512], dt.float32, tag="mm")
                    for ci in range(2):
                        nc.tensor.matmul(
                            pm[:],
                            w1_sb[:, ci, 128 * mj:128 * mj + 128],
                            h_t[:, ci, nsl_],
                            start=(ci == 0),
                            stop=(ci == 1),
                        )
                    if tailb and mj == 0:
                        nc.vector.tensor_scalar_max(o_t[:, nsl_], pm[:], 0.0)
                    else:
                        nc.scalar.activation(o_t[:, nsl_], pm[:], ACTF.Relu, bias=0.0)
                (nc.scalar if tailb else nc.sync).dma_start(
                    out_h.ap()[b, 128 * mj:128 * mj + 128, 128 * c0:128 * c0 + qn],
                    o_t[:],
                )

        # ---- phase schedule: keep the DVE scan stream dense; batch 0's
        # gather/MLP work executes under batch 1's scans.
        do_prep(0)
        do_coarse(0)
        do_gstage(0)
        do_prep(1)
        do_coarse(1)
        do_gstage(1)
        for b in range(nb):
            for gi in range(len(GROUPS[b])):
                last = (b == nb - 1 and gi >= len(GROUPS[b]) - 2)
                do_group(b, gi, mul_on_dve=last)

    nc.compile()
    return nc


_NC_CACHE = {}


def _get_nc(nb=NB):
    if nb not in _NC_CACHE:
        _NC_CACHE[nb] = build_nc(nb)
    return _NC_CACHE[nb]


def kernel(**inputs):
    from concourse.bass_utils import run_bass_kernel_spmd

    nc = _get_nc(NB)
    per_core = B_FULL // N_CORES
    in_maps = []
    for c in range(N_CORES):
        sl = slice(per_core * c, per_core * (c + 1))
        in_maps.append(
            {
                "unknown": np.ascontiguousarray(np.asarray(inputs["unknown"][sl], dtype=np.float32)),
                "known": np.ascontiguousarray(np.asarray(inputs["known"][sl], dtype=np.float32)),
                "unknow_feats": np.ascontiguousarray(np.asarray(inputs["unknow_feats"][sl], dtype=np.float32)),
                "known_feats": np.ascontiguousarray(np.asarray(inputs["known_feats"][sl], dtype=np.float32)),
                "W0": np.asarray(inputs["W0"], dtype=np.float32),
                "W1": np.asarray(inputs["W1"], dtype=np.float32),
            }
        )
    res = run_bass_kernel_spmd(nc, in_maps, core_ids=list(range(N_CORES)))
    out = np.concatenate([res.results[c]["out"] for c in range(N_CORES)], axis=0)
    return out.astype(np.float32)


# revision 53
# speedup vs baseline: 1.0831x; 1.0419x over previous
# PointNet++ feature-propagation (three_nn + three_interpolate + shared MLP)
# Trainium2 Bass/Tile kernel, 8 NeuronCores, data-parallel over batch.
#
# Per batch (n=4096 unknown, m=1024 known, C2=512, C1=256):
#  1) ONE bf16 matmul (K=24 rows of triple-bf16 splits) computes
#     D' = 2u.k - |k|^2 - |u|^2 = -d2 directly in PSUM (error ~2e-6 abs);
#     Max/MaxIndex scan PSUM directly -> top-3 neighbors + their -d2.
#  2) weights from -d2 (clamped), no refine pass needed.
#  3) G = W0a^T @ known_feats is precomputed per batch ([256, m]); its
#     transpose rows (bf16, 512B) are staged to HBM and gathered per point
#     (3 neighbors in one DGE gather), halving gather bytes vs raw feats.
#  4) interp contribution = sum_k w_k * g_k is injected into the MLP1 PSUM
#     via identity matmuls; MLP1 = relu(W0b^T uf + inject), MLP2 as usual.
#
# The two batches are software-pipelined and the post-scan work is cut into
# quarter-batch (1024-point) groups so the DVE (which owns the unavoidable
# Max/MaxIndex scans, ~150us) never stalls and the post-scan tail is short.
import numpy as np
from contextlib import ExitStack

import concourse.bass as bass
import concourse.bacc as bacc
import concourse.tile as tile
import concourse.mybir as mybir
from concourse.masks import make_identity

AP = bass.AP
dt = mybir.dt
Alu = mybir.AluOpType
ACTF = mybir.ActivationFunctionType

B_FULL = 16
N_CORES = 8
NB = 2            # batches per core
N = 4096
M = 1024
C1 = 256
C2 = 512
D0 = 256
D1 = 256

NCH = N // 128    # 32 i-chunks
MCH = M // 128    # 8 j-chunks
QCH = 8           # i-chunks per group (quarter batch)
QN = QCH * 128    # 1024 points per group
NG = NCH // QCH   # 4 groups per batch
KROWS = 24
D2_FLOOR = 1e-7   # clamp for d2 (reference adds 1e-8; matmul err ~2e-6)
IDX_REPLICATE = True  # replicate idx wrap-table to 128 partitions (HW DGE req?)


def _bf16_split3(ve, pool, x_ap, shape, tagp):
    """bf16 (hi, lo, mid) with hi+lo+mid ~= x."""
    xh = pool.tile(list(shape), dt.bfloat16, tag=tagp + "_h")
    xl = pool.tile(list(shape), dt.bfloat16, tag=tagp + "_l")
    xm = pool.tile(list(shape), dt.bfloat16, tag=tagp + "_m")
    r1 = pool.tile(list(shape), dt.float32, tag=tagp + "_r1")
    r2 = pool.tile(list(shape), dt.float32, tag=tagp + "_r2")
    ve.tensor_copy(xh[:], x_ap)
    ve.tensor_sub(r1[:], x_ap, xh[:])
    ve.tensor_copy(xl[:], r1[:])
    ve.tensor_sub(r2[:], r1[:], xl[:])
    ve.tensor_copy(xm[:], r2[:])
    return xh, xl, xm


def _v(t_ap, dims, off=0):
    """AP over t_ap's tensor with explicit [stride, count] dims (dims[0] = partition dim)."""
    return AP(t_ap.tensor, t_ap.offset + off, dims)


def build_nc(nb=NB):
    nc = bacc.Bacc("TRN2", target_bir_lowering=False, debug=False)

    unknown_h = nc.dram_tensor("unknown", [nb, N, 3], dt.float32, kind="ExternalInput")
    known_h = nc.dram_tensor("known", [nb, M, 3], dt.float32, kind="ExternalInput")
    uf_h = nc.dram_tensor("unknow_feats", [nb, C1, N], dt.float32, kind="ExternalInput")
    kf_h = nc.dram_tensor("known_feats", [nb, C2, M], dt.float32, kind="ExternalInput")
    w0_h = nc.dram_tensor("W0", [C1 + C2, D0], dt.float32, kind="ExternalInput")
    w1_h = nc.dram_tensor("W1", [D0, D1], dt.float32, kind="ExternalInput")
    out_h = nc.dram_tensor("out", [nb, D1, N], dt.float32, kind="ExternalOutput")

    GROUPS = [[(0, 8), (8, 8), (16, 8), (24, 8)] for _ in range(nb)]
    GROUPS[nb - 1] = [(0, 8), (8, 8), (16, 8), (24, 4), (28, 4)]

    gt_h = [nc.dram_tensor(f"gt{b}", [M, D0], dt.bfloat16) for b in range(nb)]
    wr_h = [[nc.dram_tensor(f"wr{b}_{gi}", [3 * sz * 128], dt.bfloat16)
             for gi, (c0, sz) in enumerate(GROUPS[b])]
            for b in range(nb)]

    with tile.TileContext(nc) as tc, ExitStack() as ctx:
        const = ctx.enter_context(tc.tile_pool(name="const", bufs=1))
        kfp = ctx.enter_context(tc.tile_pool(name="kfp", bufs=2))
        kf32p = ctx.enter_context(tc.tile_pool(name="kf32p", bufs=1))
        gtp = ctx.enter_context(tc.tile_pool(name="gtp", bufs=1))
        prep = ctx.enter_context(tc.tile_pool(name="prep", bufs=2))
        sp = ctx.enter_context(tc.tile_pool(name="split", bufs=2))
        sel = ctx.enter_context(tc.tile_pool(name="sel", bufs=2))
        wts = ctx.enter_context(tc.tile_pool(name="wts", bufs=4))
        wts4 = ctx.enter_context(tc.tile_pool(name="wts4", bufs=2))
        gat = ctx.enter_context(tc.tile_pool(name="gat", bufs=2))
        gat4 = ctx.enter_context(tc.tile_pool(name="gat4", bufs=2))
        gwp = ctx.enter_context(tc.tile_pool(name="gwp", bufs=2))
        gwp4 = ctx.enter_context(tc.tile_pool(name="gwp4", bufs=2))
        wbp = ctx.enter_context(tc.tile_pool(name="wbp", bufs=2))
        wbp4 = ctx.enter_context(tc.tile_pool(name="wbp4", bufs=2))
        mlpp = ctx.enter_context(tc.tile_pool(name="mlpp", bufs=2))
        mlpp4 = ctx.enter_context(tc.tile_pool(name="mlpp4", bufs=2))
        ps_d = ctx.enter_context(tc.tile_pool(name="ps_d", bufs=2, space="PSUM"))
        ps_mm = ctx.enter_context(tc.tile_pool(name="ps_mm", bufs=2, space="PSUM"))
        ps_trb = ctx.enter_context(tc.tile_pool(name="ps_trb", bufs=1, space="PSUM"))
        ps_tru = ctx.enter_context(tc.tile_pool(name="ps_tru", bufs=1, space="PSUM"))

        # ---------------- constants ----------------
        ident_b = const.tile([128, 128], dt.bfloat16, tag="idb")
        make_identity(nc, ident_b[:])
        ident_u = const.tile([128, 128], dt.float16, tag="idu")
        make_identity(nc, ident_u[:])

        w0_sb = const.tile([128, 6, D0], dt.bfloat16, tag="w0")
        w1_sb = const.tile([128, 2, D1], dt.bfloat16, tag="w1")
        for ci in range(6):
            wtmp = kf32p.tile([128, D0], dt.float32, tag="wstg")
            nc.sync.dma_start(wtmp[:], w0_h.ap()[128 * ci:128 * ci + 128, :])
            nc.scalar.copy(w0_sb[:, ci, :], wtmp[:])
        for ci in range(2):
            wtmp = kf32p.tile([128, D1], dt.float32, tag="wstg")
            nc.sync.dma_start(wtmp[:], w1_h.ap()[128 * ci:128 * ci + 128, :])
            nc.scalar.copy(w1_sb[:, ci, :], wtmp[:])

        lhs_alls, rhs_alls = [], []
        valls, mialls = [], []
        wtiles = {}

        def do_prep(b):
            ve = nc.gpsimd
            cpe = nc.vector.tensor_copy if b == 0 else nc.scalar.copy
            # ---- known prep -> rhs_all
            kw = prep.tile([128, MCH, 3], dt.float32, tag="kw")
            nc.sync.dma_start(
                kw[:], AP(known_h, b * M * 3, [[3, 128], [3 * 128, MCH], [1, 3]])
            )
            k2 = prep.tile([128, MCH, 3], dt.float32, tag="k2")
            ve.tensor_scalar_mul(k2[:], kw[:], 2.0)
            k2h, k2l, k2m = _bf16_split3(ve, sp, k2[:], [128, MCH, 3], "k2")
            sq = prep.tile([128, MCH, 3], dt.float32, tag="ksq")
            nc.scalar.square(sq[:], kw[:])
            s_f = prep.tile([128, MCH], dt.float32, tag="ks")
            ve.tensor_add(s_f[:], sq[:, :, 0], sq[:, :, 1])
            ve.tensor_add(s_f[:], s_f[:], sq[:, :, 2])
            ns = prep.tile([128, MCH], dt.float32, tag="kns")
            ve.tensor_scalar_mul(ns[:], s_f[:], -1.0)
            nsh, nsl, nsm = _bf16_split3(ve, sp, ns[:], [128, MCH], "kns")

            # rows: 0-2 uh|2kh, 3-5 uh|2kl, 6-8 ul|2kh, 9-11 ul|2kl,
            #       12-14 uh|2km, 15-17 um|2kh, 18-20 1|-(skh,skl,skm),
            #       21-23 -(suh,sul,sum)|1
            kch = prep.tile([128, MCH, 32], dt.bfloat16, tag="kch")
            for (r0, src) in ((0, k2h), (3, k2l), (6, k2h), (9, k2l), (12, k2m), (15, k2h)):
                cpe(kch[:, :, r0:r0 + 3], src[:])
            cpe(kch[:, :, 18], nsh[:])
            cpe(kch[:, :, 19], nsl[:])
            cpe(kch[:, :, 20], nsm[:])
            ve.memset(kch[:, :, 21:24], 1.0)
            rhs_all = prep.tile([KROWS, M], dt.bfloat16, tag="rhs_all")
            for t in range(MCH):
                pst = ps_trb.tile([48, 128], dt.bfloat16, tag="trb")
                nc.tensor.transpose(pst[:KROWS, :], kch[:, t, :KROWS], ident_b[:])
                cpe(rhs_all[:, 128 * t:128 * t + 128], pst[:KROWS, :])

            # ---- unknown prep -> lhs_all
            uw = prep.tile([128, NCH, 3], dt.float32, tag="uw")
            nc.sync.dma_start(
                uw[:], AP(unknown_h, b * N * 3, [[3, 128], [3 * 128, NCH], [1, 3]])
            )
            uh, ul, um = _bf16_split3(ve, sp, uw[:], [128, NCH, 3], "u")
            usq = prep.tile([128, NCH, 3], dt.float32, tag="usq")
            nc.scalar.square(usq[:], uw[:])
            su = prep.tile([128, NCH], dt.float32, tag="us")
            ve.tensor_add(su[:], usq[:, :, 0], usq[:, :, 1])
            ve.tensor_add(su[:], su[:], usq[:, :, 2])
            nsu = prep.tile([128, NCH], dt.float32, tag="uns")
            ve.tensor_scalar_mul(nsu[:], su[:], -1.0)
            nsuh, nsul, nsum_ = _bf16_split3(ve, sp, nsu[:], [128, NCH], "uns")

            uch = prep.tile([128, NCH, 32], dt.bfloat16, tag="uch")
            for (r0, src) in ((0, uh), (3, uh), (6, ul), (9, ul), (12, uh), (15, um)):
                cpe(uch[:, :, r0:r0 + 3], src[:])
            ve.memset(uch[:, :, 18:21], 1.0)
            cpe(uch[:, :, 21], nsuh[:])
            cpe(uch[:, :, 22], nsul[:])
            cpe(uch[:, :, 23], nsum_[:])
            lhs_all = prep.tile([KROWS, N], dt.bfloat16, tag="lhs_all")
            for t in range(NCH):
                pst = ps_trb.tile([48, 128], dt.bfloat16, tag="trb")
                nc.tensor.transpose(pst[:KROWS, :], uch[:, t, :KROWS], ident_b[:])
                (cpe if t == 0 else nc.scalar.copy)(
                    lhs_all[:, 128 * t:128 * t + 128], pst[:KROWS, :])
            lhs_alls.append(lhs_all)
            rhs_alls.append(rhs_all)

        def do_weights(b, gi, c0, sz):
            # weights + idx list right after this group's scans. For non-tail
            # groups, keep the DVE scan stream clean: only the reciprocals
            # (DVE-exclusive) stay on DVE; the rest goes to Pool/Act.
            vall, miall = valls[b], mialls[b]
            tail = b == nb - 1 and gi >= len(GROUPS[b]) - 2
            ew = nc.vector if tail else nc.gpsimd
            cw = nc.vector.tensor_copy if tail else nc.scalar.copy
            wp = wts if sz == QCH else wts4
            tsl = slice(c0, c0 + sz)
            d23 = wp.tile([128, sz, 3], dt.float32, tag="d23")
            ew.tensor_scalar(
                d23[:], vall[:, tsl, 0:3], -1.0, D2_FLOOR, op0=Alu.mult, op1=Alu.max
            )
            r3 = wp.tile([128, sz, 3], dt.float32, tag="r3")
            nc.vector.reciprocal(r3[:], d23[:])
            z = wp.tile([128, sz], dt.float32, tag="z")
            ew.tensor_add(z[:], r3[:, :, 0], r3[:, :, 1])
            ew.tensor_add(z[:], z[:], r3[:, :, 2])
            iz = wp.tile([128, sz], dt.float32, tag="iz")
            nc.vector.reciprocal(iz[:], z[:])
            w3f = wp.tile([128, sz, 3], dt.float32, tag="w3f")
            ew.tensor_mul(w3f[:], r3[:], iz[:].to_broadcast([128, sz, 3]))
            # k-major bf16: w3b3[p, k*sz + t] = w3f[p, t, k]
            w3b3 = wp.tile([128, 3 * sz], dt.bfloat16, tag="w3b3")
            cw(
                _v(w3b3[:], [w3b3[:].ap[0], [1, sz], [sz, 3]]),
                w3f[:],
            )
            # k-major fp16 neighbor indices
            j3h3 = wp.tile([128, 3 * sz], dt.float16, tag="j3h3")
            cw(
                _v(j3h3[:], [j3h3[:].ap[0], [1, sz], [sz, 3]]),
                miall[:, tsl, 0:3],
            )
            wtiles[(b, gi)] = (w3b3, j3h3)

        def do_coarse(b):
            vall = sel.tile([128, NCH, 8], dt.float32, tag="vall")
            miall = sel.tile([128, NCH, 8], dt.uint16, tag="miall")
            valls.append(vall)
            mialls.append(miall)
            lhs_all, rhs_all = lhs_alls[b], rhs_alls[b]
            for t in range(NCH):
                psd = ps_d.tile([128, 1024], dt.float32, tag="psd")
                for hm in range(2):
                    nc.tensor.matmul(
                        psd[:, 512 * hm:512 * hm + 512],
                        lhs_all[:, 128 * t:128 * t + 128],
                        rhs_all[:, 512 * hm:512 * hm + 512],
                        start=True,
                        stop=True,
                    )
                nc.vector.max(out=vall[:, t, :], in_=psd[:])
                nc.vector.max_index(
                    out=miall[:, t, :], in_max=vall[:, t, :], in_values=psd[:]
                )
                for gi, (c0, sz) in enumerate(GROUPS[b]):
                    if t == c0 + sz - 1:
                        do_weights(b, gi, c0, sz)

        def do_gstage(b):
            # G = W0a^T @ KF -> G^T rows (bf16) staged to HBM
            kf16 = kfp.tile([128, 4, M], dt.bfloat16, tag="kf16")
            for cj in range(4):
                kf32 = kf32p.tile([128, M], dt.float32, tag="kf32")
                nc.sync.dma_start(kf32[:], kf_h.ap()[b, 128 * cj:128 * cj + 128, :])
                nc.scalar.copy(kf16[:, cj, :], kf32[:])
            gtsb = gtp.tile([128, MCH, D0], dt.bfloat16, tag="gtsb")
            for mt in range(MCH):
                pg = ps_mm.tile([128, 512], dt.float32, tag="mm")
                for cj in range(4):
                    nc.tensor.matmul(
                        pg[:, 0:D0],
                        kf16[:, cj, 128 * mt:128 * mt + 128],
                        w0_sb[:, cj, :],
                        start=(cj == 0),
                        stop=(cj == 3),
                    )
                nc.scalar.copy(gtsb[:, mt, :], pg[:, 0:D0])
            nc.sync.dma_start(
                _v(gt_h[b].ap(), [[D0, 128], [128 * D0, MCH], [1, D0]]),
                gtsb[:],
            )

        def do_group(b, gi, mul_on_dve):
            c0, sz = GROUPS[b][gi]
            qn = sz * 128
            big = sz == QCH
            tailb = b == nb - 1 and gi >= len(GROUPS[b]) - 2
            wp = wts if big else wts4
            gatp = gat if big else gat4
            gwpp = gwp if big else gwp4
            wbpp = wbp if big else wbp4
            mpp = mlpp if big else mlpp4
            w3b3, j3h3 = wtiles[(b, gi)]
            # --- weight broadcast row via HBM round-trip
            pswt = ps_trb.tile([48, 128], dt.bfloat16, tag="trb")
            nc.tensor.transpose(pswt[:3 * sz, :], w3b3[:], ident_b[:])
            wsb3 = wp.tile([3 * sz, 128], dt.bfloat16, tag="wsb3")
            nc.scalar.copy(wsb3[:], pswt[:3 * sz, :])
            nc.sync.dma_start(
                _v(wr_h[b][gi].ap(), [[128, 3 * sz], [1, 128]]),
                wsb3[:],
            )
            wb3 = wbpp.tile([128, 3 * qn], dt.bfloat16, tag="wb3")

            # --- idxw3: wrap-16 layout of the 3 neighbor index lists
            psj = ps_tru.tile([48, 384], dt.float16, tag="trj")
            nc.tensor.transpose(psj[:3 * sz, 0:128], j3h3[:], ident_u[:])
            mit3 = wp.tile([3 * sz, 128], dt.float16, tag="mit3")
            (nc.vector.tensor_copy if tailb else nc.scalar.copy)(mit3[:], psj[:3 * sz, 0:128])
            idxw3 = wp.tile([128, 3 * qn // 16], dt.int16, tag="idxw3")
            # all 8 fold-transposes write disjoint column ranges of ONE psum
            # tile (no WAR), then a single strided copy extracts the wrap-16
            # index table: pst2[p16, s*3sz + k*sz + t] -> idxw3[p16, k*(qn//16)+t*8+s]
            pst2 = ps_tru.tile([48, 384], dt.float16, tag="trj")
            for s in range(8):
                nc.tensor.transpose(
                    pst2[:16, 3 * sz * s:3 * sz * s + 3 * sz],
                    mit3[:, 16 * s:16 * s + 16],
                    ident_u[:3 * sz, :3 * sz]
                )
            (nc.vector.tensor_copy if tailb else nc.scalar.copy)(
                _v(idxw3[:16, :],
                   [idxw3[:16, :].ap[0], [1, 8], [qn // 16, 3], [8, sz]]),
                _v(pst2[:16, :],
                   [pst2[:16, :].ap[0], [3 * sz, 8], [sz, 3], [1, sz]]),
            )
            if IDX_REPLICATE:
                for r in range(1, 8):
                    if tailb:
                        eng = nc.sync if r % 2 == 0 else nc.scalar
                    else:
                        eng = nc.gpsimd
                    eng.dma_start(idxw3[16 * r:16 * r + 16, :], idxw3[0:16, :])
            (nc.sync if tailb else nc.scalar).dma_start(
                wb3[:], AP(wr_h[b][gi], 0, [[0, 128], [1, 3 * qn]])
            )

            # --- one gather for all 3 neighbors (channel-major bf16)
            g3 = gatp.tile([128, 2, 3 * qn], dt.bfloat16, tag="g3")
            nc.gpsimd.dma_gather(
                g3[:],
                gt_h[b].ap(),
                idxw3[:],
                3 * qn,
                3 * qn,
                D0,
                transpose=True,
                single_packet=False,
            )
            # --- weight multiply (Pool; last group splits with idle DVE)
            gw3 = gwpp.tile([128, 2, 3 * qn], dt.bfloat16, tag="gw3")
            wbb = _v(wb3[:], [wb3[:].ap[0], [0, 2], [1, 3 * qn]])
            if mul_on_dve:
                nc.vector.tensor_mul(gw3[:, 0, :], g3[:, 0, :], wb3[:])
                nc.vector.tensor_mul(gw3[:, 1, :], g3[:, 1, :], wb3[:])
            else:
                nc.gpsimd.tensor_mul(gw3[:], g3[:], wbb)

            # --- unknow_feats -> bf16
            uf16 = mpp.tile([128, 2, qn], dt.bfloat16, tag="uf16")
            for cj in range(2):
                uf32 = mpp.tile([128, qn], dt.float32, tag="uf32")
                nc.sync.dma_start(
                    uf32[:],
                    uf_h.ap()[b, 128 * cj:128 * cj + 128, 128 * c0:128 * c0 + qn],
                )
                (nc.gpsimd.tensor_copy if tailb else nc.scalar.copy)(uf16[:, cj, :], uf32[:])

            # --- MLP1: relu(W0b^T uf + sum_k inject(gw3_k)) -> h_t bf16
            h_t = mpp.tile([128, 2, qn], dt.bfloat16, tag="h")
            for mj in range(2):
                for nci in range(qn // 512):
                    nsl_ = slice(512 * nci, 512 * nci + 512)
                    pm = ps_mm.tile([128, 512], dt.float32, tag="mm")
                    for ci in range(2):
                        nc.tensor.matmul(
                            pm[:],
                            w0_sb[:, 4 + ci, 128 * mj:128 * mj + 128],
                            uf16[:, ci, nsl_],
                            start=(ci == 0),
                            stop=False,
                        )
                    for k in range(3):
                        nc.tensor.matmul(
                            pm[:],
                            ident_b[:],
                            gw3[:, mj, k * qn + 512 * nci:k * qn + 512 * nci + 512],
                            start=False,
                            stop=(k == 2),
                        )
                    if tailb and mj == 0:
                        nc.vector.tensor_scalar_max(h_t[:, mj, nsl_], pm[:], 0.0)
                    else:
                        nc.scalar.activation(h_t[:, mj, nsl_], pm[:], ACTF.Relu, bias=0.0)

            # --- MLP2 (relu) -> fp32 out
            for mj in range(2):
                o_t = mpp.tile([128, qn], dt.float32, tag="o")
                for nci in range(qn // 512):
                    nsl_ = slice(512 * nci, 512 * nci + 512)
                    pm = ps_mm.tile([128, 512], dt.float32, tag="mm")
                    for ci in range(2):
                        nc.tensor.matmul(
                            pm[:],
                            w1_sb[:, ci, 128 * mj:128 * mj + 128],
                            h_t[:, ci, nsl_],
                            start=(ci == 0),
                            stop=(ci == 1),
                        )
                    if tailb and mj == 0:
                        nc.vector.tensor_scalar_max(o_t[:, nsl_], pm[:], 0.0)
                    else:
                        nc.scalar.activation(o_t[:, nsl_], pm[:], ACTF.Relu, bias=0.0)
                (nc.scalar if tailb else nc.sync).dma_start(
                    out_h.ap()[b, 128 * mj:128 * mj + 128, 128 * c0:128 * c0 + qn],
                    o_t[:],
                )

        # ---- phase schedule: keep the DVE scan stream dense; batch 0's
        # gather/MLP work executes under batch 1's scans.
        do_prep(0)
        do_coarse(0)
        do_gstage(0)
        do_prep(1)
        do_coarse(1)
        do_gstage(1)
        for b in range(nb):
            for gi in range(len(GROUPS[b])):
                last = (b == nb - 1 and gi >= len(GROUPS[b]) - 2)
                do_group(b, gi, mul_on_dve=last)

    nc.compile()
    return nc


_NC_CACHE = {}


def _get_nc(nb=NB):
    if nb not in _NC_CACHE:
        _NC_CACHE[nb] = build_nc(nb)
    return _NC_CACHE[nb]


def kernel(**inputs):
    from concourse.bass_utils import run_bass_kernel_spmd

    nc = _get_nc(NB)
    per_core = B_FULL // N_CORES
    in_maps = []
    for c in range(N_CORES):
        sl = slice(per_core * c, per_core * (c + 1))
        in_maps.append(
            {
                "unknown": np.ascontiguousarray(np.asarray(inputs["unknown"][sl], dtype=np.float32)),
                "known": np.ascontiguousarray(np.asarray(inputs["known"][sl], dtype=np.float32)),
                "unknow_feats": np.ascontiguousarray(np.asarray(inputs["unknow_feats"][sl], dtype=np.float32)),
                "known_feats": np.ascontiguousarray(np.asarray(inputs["known_feats"][sl], dtype=np.float32)),
                "W0": np.asarray(inputs["W0"], dtype=np.float32),
                "W1": np.asarray(inputs["W1"], dtype=np.float32),
            }
        )
    res = run_bass_kernel_spmd(nc, in_maps, core_ids=list(range(N_CORES)))
    out = np.concatenate([res.results[c]["out"] for c in range(N_CORES)], axis=0)
    return out.astype(np.float32)
